# revision 29
# baseline (speedup 1.0000x reference)
"""Multi-head attention (B=8, S=1024, D=1024, H=16) on 8 TRN2 NeuronCores.

Sharding: pure data parallel - batch element b on core b. Weights are
broadcast to every core. No collectives.

Per-core pipeline (X: [S, D] for one batch element), restructured so the
softmax exp (the ACT-engine bottleneck) overlaps PE matmul work:

  A. X^T via PE transposes (bf16 matmul against identity), 4 blocks per
     PSUM bank, one wide ACT copy per 4 blocks.
  B(pr). Q^T/K^T projection for head pair pr only (nt = pr and 8+pr),
     dk-outer / sc-inner loop so each LDWEIGHTS feeds 2 matmuls; bias
     (+psum->bf16 copy) on DVE to keep ACT free for exp.
  C. V = X @ W_v (natural layout) into V_aug[sk, head, 65] with a ones
     column (col 64) that makes the PV matmul accumulate the softmax
     denominator in row 64.
  D(pr). attention for the pair: scores^T = K_h^T.T @ Q_h^T into a wide
     [128,1024] PSUM tile (two heads row-split, concurrent on the PE),
     exp split 6:2 between ACT (exact, scale=1/8) and DVE (Schraudolph
     int16 bit-trick writing bf16 bit patterns), PV with the ones row.
     Normalize is decoupled from the PSUM path: one bf16 copy per head
     frees the pv bank, then reciprocal_approx_fast (from a partition-0
     staging row - custom-DVE ops mis-read nonzero base partitions on
     HW) + GpSimd partition-broadcast + DVE multiply -> attnT bf16.
  E. Y = attn_out @ W_out + b_out in bf16.

Emission order A; B(0); D(0,0){scores; C as defer hook; PV}; D(0,1);
B(1); D(1); ... B(7); D(7); E lets the Tile scheduler run B(pr+1)
matmuls in the PE gaps of D(pr) while ACT/DVE stream the exps, and run
D(0)'s scores/exps during C. PSUM: 2x[128,512] misc + 2x[128,1024]
scores + 2x[128,512] pv = 8 banks.
"""

import sys

sys.path.insert(0, "/opt/trn_rl_repo")

import numpy as np

import concourse.bacc as bacc
import concourse.mybir as mybir
from concourse.bass_utils import run_bass_kernel_spmd
from concourse.masks import make_identity
from concourse.tile import TileContext

B = 8
S = 1024
D = 1024
H = 16
DK = D // H  # 64
P = 128
ST = S // P   # 8 s-tiles
DT = D // P   # 8 d-tiles
NTQK = 2 * D // P  # 16 n-tiles for the Q|K part
PAIRS = H // 2     # 8 head pairs
SC = S // 512      # 2 chunks of 512 (matmul free-dim limit)

f32 = mybir.dt.float32
bf16 = mybir.dt.bfloat16
i16 = mybir.dt.int16
EXP = mybir.ActivationFunctionType.Exp
MULT = mybir.AluOpType.mult
ADD = mybir.AluOpType.add

# Schraudolph exp on DVE: bf16 bits of exp(s/8) = int16((a/(8*65536))*s + b)
# with a = 2^23/ln2, b = (127*2^23 - C)/65536, C = 366393 (min max-rel-err).
SCH_A = float(2.0**23 / np.log(2.0) / (8.0 * 65536.0))
SCH_B = float((127 * 2.0**23 - 366393.0) / 65536.0)
# which sk-tiles of each (pair, sc) block use the DVE exp (rest use ACT)
DVE_EXP_SKS = frozenset({3, 7})


def build_nc():
    nc = bacc.Bacc()
    X = nc.dram_tensor("X", [S, D], f32, kind="ExternalInput")
    W_in = nc.dram_tensor("W_in", [D, 3 * D], f32, kind="ExternalInput")
    b_in = nc.dram_tensor("b_in", [3 * D], f32, kind="ExternalInput")
    W_out = nc.dram_tensor("W_out", [D, D], f32, kind="ExternalInput")
    b_out = nc.dram_tensor("b_out", [D], f32, kind="ExternalInput")
    out = nc.dram_tensor("out", [S, D], f32, kind="ExternalOutput")

    w_in_kp = W_in.rearrange("(ko p) n -> p ko n", p=P)  # [128, 8, 3072]
    w_out_kp = W_out.rearrange("(ko p) n -> p ko n", p=P)  # [128, 8, 1024]

    with TileContext(nc) as tc:
        const = tc.alloc_tile_pool(name="const", bufs=1)
        # PSUM: mps 2x[128,512] (2 banks) shared by A/B/C/E, dwide
        # 2x[128,1024] (4 banks) for scores, pvp 2x[128,512] (2 banks).
        mps = tc.alloc_tile_pool(name="mps", bufs=2, space="PSUM")
        dwide = tc.alloc_tile_pool(name="dwide", bufs=2, space="PSUM")
        pvp = tc.alloc_tile_pool(name="pvp", bufs=2, space="PSUM")

        identity = const.tile([P, P], bf16)
        make_identity(nc, identity[:])
        bqk = const.tile([P, NTQK], f32)
        nc.sync.dma_start(bqk[:], b_in[0 : 2 * D].rearrange("(o p) -> p o", p=P))
        bv_bc = const.tile([P, D], f32)
        bout_bc = const.tile([P, D], f32)
        ones4 = const.tile([P, ST, H, 1], f32)
        nc.vector.memset(ones4[:], 1.0)

        # ---------------- resident tensors ----------------
        qkT_pool = tc.alloc_tile_pool(name="qkT", bufs=1)
        qkT = qkT_pool.tile([P, NTQK, S], bf16)  # 4 MB
        vaug_pool = tc.alloc_tile_pool(name="vaug", bufs=1)
        v_aug = vaug_pool.tile([P, ST, H, DK + 1], bf16)  # 2.1 MB
        nc.vector.tensor_copy(v_aug[:, :, :, DK : DK + 1], ones4[:])
        xT_pool = tc.alloc_tile_pool(name="xT", bufs=1)
        xT = xT_pool.tile([P, DT, S], bf16)  # 2 MB
        attnT_pool = tc.alloc_tile_pool(name="attnT", bufs=1)
        attnT_lo = attnT_pool.tile([P, DT // 2, S], bf16)  # pairs 0-3
        attnT_hi = attnT_pool.tile([P, DT // 2, S], bf16)  # pairs 4-7
        ypart_pool = tc.alloc_tile_pool(name="ypart", bufs=1)
        # partial output projection over pairs 0-3 (+b_out), built during
        # pairs 4-7 so the E tail only covers the other half
        y_part = ypart_pool.tile([P, ST, D], bf16)  # 2 MB
        woutb_pool = tc.alloc_tile_pool(name="woutb", bufs=1)
        wout_bf = woutb_pool.tile([P, DT, D], bf16)  # 2 MB
        wv_pool = tc.alloc_tile_pool(name="wv", bufs=1)
        wv_tile = wv_pool.tile([P, DT, D], bf16)  # 2 MB

        pa = tc.alloc_tile_pool(name="phaseA", bufs=1)
        bv_row = pa.tile([1, D], f32)
        nc.sync.dma_start(bv_row[:], b_in[None, 2 * D : 3 * D])
        nc.gpsimd.partition_broadcast(bv_bc[:], bv_row[:])
        bout_row = pa.tile([1, D], f32)
        nc.sync.dma_start(bout_row[:], b_out[None, :])
        nc.gpsimd.partition_broadcast(bout_bc[:], bout_row[:])

        wqk = tc.alloc_tile_pool(name="wqk", bufs=3)

        def emit_w_dma(nt):
            w_stage = wqk.tile([P, DT, P], f32, tag="ws")
            nc.sync.dma_start(w_stage[:], w_in_kp[:, :, nt * P : (nt + 1) * P])
            return w_stage

        # ---------------- phase A: X^T (PE transpose, cast to bf16) --------
        w_pref = {}
        with tc.tile_pool(name="xstage", bufs=2) as xstage:
            for si in range(ST):
                x_tile = xstage.tile([P, D], f32, tag="x")
                nc.sync.dma_start(x_tile[:], X[si * P : (si + 1) * P, :])
                if si == ST - 1:
                    # pair-0 QK weights queue right behind the X transfers
                    w_pref = {0: emit_w_dma(0), PAIRS: emit_w_dma(PAIRS)}
                xb = xstage.tile([P, D], bf16, tag="xb")
                nc.vector.tensor_copy(xb[:], x_tile[:])
                for half in range(2):
                    tp = mps.tile([P, 512], f32, tag="w", name="tp")
                    for j in range(4):
                        dj = half * 4 + j
                        nc.tensor.matmul(
                            tp[:, j * P : (j + 1) * P],
                            xb[:, dj * P : (dj + 1) * P],
                            identity[:],
                            start=True,
                            stop=True,
                        )
                    nc.scalar.copy(
                        xT[:, half * 4 : (half + 1) * 4, si * P : (si + 1) * P],
                        tp[:].rearrange("p (j q) -> p j q", q=P),
                    )

        # ---------------- per-pair B (QK projection) ----------------
        def emit_b(pr):
            for nt in (pr, PAIRS + pr):
                w_stage = w_pref.pop(nt, None)
                if w_stage is None:
                    w_stage = emit_w_dma(nt)
                w_tile = wqk.tile([P, DT, P], bf16, tag="w")
                nc.vector.tensor_copy(w_tile[:], w_stage[:])
                ps = [
                    mps.tile([P, 512], f32, tag="w", name=f"psb{sc}")
                    for sc in range(SC)
                ]
                for dk in range(DT):
                    for sc in range(SC):
                        nc.tensor.matmul(
                            ps[sc][:],
                            w_tile[:, dk, :],
                            xT[:, dk, sc * 512 : (sc + 1) * 512],
                            start=(dk == 0),
                            stop=(dk == DT - 1),
                        )
                for sc in range(SC):
                    nc.vector.tensor_scalar(
                        qkT[:, nt, sc * 512 : (sc + 1) * 512],
                        ps[sc][:],
                        bqk[:, nt : nt + 1],
                        None,
                        ADD,
                    )

        # ---------------- per-pair D (attention) ----------------
        def emit_scores(pr, sc, sk, expp):
            wide = dwide.tile([P, S], f32, tag="sps", name="sps")
            for hh in range(2):
                base = hh * DK
                nc.tensor.matmul(
                    wide[:, hh * 512 : (hh + 1) * 512],
                    qkT[base : base + DK, PAIRS + pr, sk * P : (sk + 1) * P],
                    qkT[base : base + DK, pr, sc * 512 : (sc + 1) * 512],
                    start=True,
                    stop=True,
                )
            ex = expp.tile([P, S], bf16, tag="ex")
            if sk in DVE_EXP_SKS:
                nc.vector.tensor_scalar(
                    ex[:].bitcast(i16), wide[:], SCH_A, SCH_B, MULT, ADD
                )
            else:
                nc.scalar.activation(
                    ex[:], wide[:], EXP, scale=1.0 / np.sqrt(DK)
                )
            return ex

        def emit_pv(pr, sk, ex, pv):
            for hh in range(2):
                h = 2 * pr + hh
                nc.tensor.matmul(
                    pv[hh][0 : DK + 1, :],
                    v_aug[:, sk, h, :],
                    ex[:, hh * 512 : (hh + 1) * 512],
                    start=(sk == 0),
                    stop=(sk == ST - 1),
                )

        def emit_norm(pr, sc, pv, expp, bcp, rrowp):
            # Decouple normalization from the PSUM critical path: one bf16
            # copy per head frees the pv bank (values + denominator row);
            # the reciprocal chain then runs off-path from SBUF.
            upv = [
                rrowp.tile([P, 512], bf16, tag=f"upv{hh}", name=f"upv{hh}")
                for hh in range(2)
            ]
            for hh in range(2):
                nc.vector.tensor_copy(upv[hh][0 : DK + 1, :], pv[hh][0 : DK + 1, :])
            drow = rrowp.tile([1, S], f32, tag="dr", name="drow")
            for hh in range(2):
                # custom-DVE ops mis-read nonzero base_partition on HW:
                # stage the partition-64 denominator rows at partition 0
                nc.scalar.copy(
                    drow[:, hh * 512 : (hh + 1) * 512],
                    upv[hh][DK : DK + 1, :],
                )
            nc.vector.reciprocal_approx_fast(drow[:], drow[:])
            # full-tile broadcast (sliced outputs break on HW)
            bc = bcp.tile([P, S], f32, tag="bc", name="bc")
            nc.gpsimd.partition_broadcast(bc[:], drow[:])
            attnT_half = attnT_lo if pr < DT // 2 else attnT_hi
            for hh in range(2):
                base = hh * DK
                nc.vector.tensor_tensor(
                    attnT_half[
                        base : base + DK, pr % (DT // 2),
                        sc * 512 : (sc + 1) * 512,
                    ],
                    upv[hh][0:DK, :],
                    bc[0:DK, hh * 512 : (hh + 1) * 512],
                    MULT,
                )

        def emit_d_block(pr, sc, expp, bcp, rrowp, defer_hook=None):
            pv_alloc = lambda: [
                pvp.tile([P, 512], f32, tag="pv", name=f"pv{i}")
                for i in range(2)
            ]
            if defer_hook is None:
                pv = pv_alloc()
                for sk in range(ST):
                    ex = emit_scores(pr, sc, sk, expp)
                    emit_pv(pr, sk, ex, pv)
            else:
                exs = [emit_scores(pr, sc, sk, expp) for sk in range(ST)]
                defer_hook()
                pv = pv_alloc()
                for sk in range(ST):
                    emit_pv(pr, sk, exs[sk], pv)
            emit_norm(pr, sc, pv, expp, bcp, rrowp)

        with (
            tc.tile_pool(name="wstage", bufs=1) as wstage,
            tc.tile_pool(name="expp", bufs=6) as expp,
            tc.tile_pool(name="bcp", bufs=2) as bcp,
            tc.tile_pool(name="rrow", bufs=2) as rrowp,
        ):

            # stage V weights right behind pair-0 QK on the DMA queue
            for ch in range(4):
                wv_stage = wstage.tile([P, DT, 256], f32, tag="wvs")
                nc.sync.dma_start(
                    wv_stage[:],
                    w_in_kp[:, :, 2 * D + ch * 256 : 2 * D + (ch + 1) * 256],
                )
                nc.vector.tensor_copy(
                    wv_tile[:, :, ch * 256 : (ch + 1) * 256], wv_stage[:]
                )
            # W_out staged load + bf16 cast (transfers overlap B/C/D)
            for ch in range(4):
                wo_stage = wstage.tile([P, DT, 256], f32, tag="wvs")
                nc.sync.dma_start(
                    wo_stage[:],
                    w_out_kp[:, :, ch * 256 : (ch + 1) * 256],
                )
                nc.vector.tensor_copy(
                    wout_bf[:, :, ch * 256 : (ch + 1) * 256], wo_stage[:]
                )

            # ---------------- phase C: V projection (split by ncx) -------
            def emit_c(ncx):
                for st in range(ST):
                    ps = mps.tile([P, 512], f32, tag="w", name="psc")
                    for dk in range(DT):
                        nc.tensor.matmul(
                            ps[:],
                            xT[:, dk, st * P : (st + 1) * P],
                            wv_tile[:, dk, ncx * 512 : (ncx + 1) * 512],
                            start=(dk == 0),
                            stop=(dk == DT - 1),
                        )
                    nc.vector.tensor_tensor(
                        v_aug[:, st, 8 * ncx : 8 * (ncx + 1), 0:DK],
                        ps[:].rearrange("p (h d) -> p h d", d=DK),
                        bv_bc[:, ncx * 512 : (ncx + 1) * 512].rearrange(
                            "p (h d) -> p h d", d=DK
                        ),
                        ADD,
                    )

            emit_b(0)
            # D(0,0) scores+exps run during C; its PVs come after C
            emit_d_block(
                0, 0, expp, bcp, rrowp,
                defer_hook=lambda: (emit_c(0), emit_c(1)),
            )
            emit_d_block(0, 1, expp, bcp, rrowp)

            # first-half output projection, spread across pairs 4-7
            def emit_e1(stp):
                for st in (2 * stp, 2 * stp + 1):
                    ps = [
                        mps.tile([P, 512], f32, tag="w", name=f"ps1{ncx}")
                        for ncx in range(SC)
                    ]
                    for dk in range(DT // 2):
                        for ncx in range(SC):
                            nc.tensor.matmul(
                                ps[ncx][:],
                                attnT_lo[:, dk, st * P : (st + 1) * P],
                                wout_bf[:, dk, ncx * 512 : (ncx + 1) * 512],
                                start=(dk == 0),
                                stop=(dk == DT // 2 - 1),
                            )
                    for ncx in range(SC):
                        nc.vector.tensor_tensor(
                            y_part[:, st, ncx * 512 : (ncx + 1) * 512],
                            ps[ncx][:],
                            bout_bc[:, ncx * 512 : (ncx + 1) * 512],
                            ADD,
                        )

            # ---------------- remaining pairs ----------------
            for pr in range(1, PAIRS):
                emit_b(pr)
                if pr >= 4:
                    emit_e1(pr - 4)
                for sc in range(SC):
                    emit_d_block(pr, sc, expp, bcp, rrowp)


        # ------------- phase E: second-half projection + y_part -------------
        with tc.tile_pool(name="ypool", bufs=2) as ypool:
            for st in range(ST):
                ps = [
                    mps.tile([P, 512], f32, tag="w", name=f"pse{ncx}")
                    for ncx in range(SC)
                ]
                for dk in range(DT // 2):
                    for ncx in range(SC):
                        nc.tensor.matmul(
                            ps[ncx][:],
                            attnT_hi[:, dk, st * P : (st + 1) * P],
                            wout_bf[
                                :, DT // 2 + dk, ncx * 512 : (ncx + 1) * 512
                            ],
                            start=(dk == 0),
                            stop=(dk == DT // 2 - 1),
                        )
                y = ypool.tile([P, D], f32, tag="y")
                for ncx in range(SC):
                    nc.vector.tensor_tensor(
                        y[:, ncx * 512 : (ncx + 1) * 512],
                        ps[ncx][:],
                        y_part[:, st, ncx * 512 : (ncx + 1) * 512],
                        ADD,
                    )
                nc.sync.dma_start(out[st * P : (st + 1) * P, :], y[:])

        for pool in (
            wqk,
            pa,
            wv_pool,
            woutb_pool,
            ypart_pool,
            attnT_pool,
            xT_pool,
            vaug_pool,
            qkT_pool,
            pvp,
            dwide,
            mps,
            const,
        ):
            pool.release()

    nc.finalize()
    return nc


_NC_CACHE = {}


def get_nc():
    if "nc" not in _NC_CACHE:
        _NC_CACHE["nc"] = build_nc()
    return _NC_CACHE["nc"]


def kernel(X, W_in, b_in, W_out, b_out):
    X = np.ascontiguousarray(np.asarray(X, dtype=np.float32))
    W_in = np.ascontiguousarray(np.asarray(W_in, dtype=np.float32))
    b_in = np.ascontiguousarray(np.asarray(b_in, dtype=np.float32))
    W_out = np.ascontiguousarray(np.asarray(W_out, dtype=np.float32))
    b_out = np.ascontiguousarray(np.asarray(b_out, dtype=np.float32))

    nc = get_nc()
    in_maps = [
        {"X": X[i], "W_in": W_in, "b_in": b_in, "W_out": W_out, "b_out": b_out}
        for i in range(B)
    ]
    res = run_bass_kernel_spmd(nc, in_maps, core_ids=list(range(B)))
    return np.stack([res.results[i]["out"] for i in range(B)], axis=0)


# revision 30
# speedup vs baseline: 1.0064x; 1.0064x over previous
"""Multi-head attention (B=8, S=1024, D=1024, H=16) on 8 TRN2 NeuronCores.

Sharding: pure data parallel - batch element b on core b. Weights are
broadcast to every core. No collectives.

Per-core pipeline (X: [S, D] for one batch element), restructured so the
softmax exp (the ACT-engine bottleneck) overlaps PE matmul work:

  A. X^T via PE transposes (bf16 matmul against identity), 4 blocks per
     PSUM bank, one wide ACT copy per 4 blocks.
  B(pr). Q^T/K^T projection for head pair pr only (nt = pr and 8+pr),
     dk-outer / sc-inner loop so each LDWEIGHTS feeds 2 matmuls; bias
     (+psum->bf16 copy) on DVE to keep ACT free for exp.
  C. V = X @ W_v (natural layout) into V_aug[sk, head, 65] with a ones
     column (col 64) that makes the PV matmul accumulate the softmax
     denominator in row 64.
  D(pr). attention for the pair: scores^T = K_h^T.T @ Q_h^T into a wide
     [128,1024] PSUM tile (two heads row-split, concurrent on the PE),
     exp split 6:2 between ACT (exact, scale=1/8) and DVE (Schraudolph
     int16 bit-trick writing bf16 bit patterns), PV with the ones row.
     Normalize is decoupled from the PSUM path: one bf16 copy per head
     frees the pv bank, then reciprocal_approx_fast (from a partition-0
     staging row - custom-DVE ops mis-read nonzero base partitions on
     HW) + GpSimd partition-broadcast + DVE multiply -> attnT bf16.
  E. Y = attn_out @ W_out + b_out in bf16.

Emission order A; B(0); D(0,0){scores; C as defer hook; PV}; D(0,1);
B(1); D(1); ... B(7); D(7); E lets the Tile scheduler run B(pr+1)
matmuls in the PE gaps of D(pr) while ACT/DVE stream the exps, and run
D(0)'s scores/exps during C. PSUM: 2x[128,512] misc + 2x[128,1024]
scores + 2x[128,512] pv = 8 banks.
"""

import sys

sys.path.insert(0, "/opt/trn_rl_repo")

import numpy as np

import concourse.bacc as bacc
import concourse.mybir as mybir
from concourse.bass_utils import run_bass_kernel_spmd
from concourse.masks import make_identity
from concourse.tile import TileContext

B = 8
S = 1024
D = 1024
H = 16
DK = D // H  # 64
P = 128
ST = S // P   # 8 s-tiles
DT = D // P   # 8 d-tiles
NTQK = 2 * D // P  # 16 n-tiles for the Q|K part
PAIRS = H // 2     # 8 head pairs
SC = S // 512      # 2 chunks of 512 (matmul free-dim limit)

f32 = mybir.dt.float32
bf16 = mybir.dt.bfloat16
i16 = mybir.dt.int16
EXP = mybir.ActivationFunctionType.Exp
MULT = mybir.AluOpType.mult
ADD = mybir.AluOpType.add

# Schraudolph exp on DVE: bf16 bits of exp(s/8) = int16((a/(8*65536))*s + b)
# with a = 2^23/ln2, b = (127*2^23 - C)/65536, C = 366393 (min max-rel-err).
SCH_A = float(2.0**23 / np.log(2.0) / (8.0 * 65536.0))
SCH_B = float((127 * 2.0**23 - 366393.0) / 65536.0)
# which sk-tiles of each (pair, sc) block use the DVE exp (rest use ACT)
DVE_EXP_SKS = frozenset({3, 7})


def build_nc():
    nc = bacc.Bacc()
    X = nc.dram_tensor("X", [S, D], f32, kind="ExternalInput")
    W_in = nc.dram_tensor("W_in", [D, 3 * D], f32, kind="ExternalInput")
    b_in = nc.dram_tensor("b_in", [3 * D], f32, kind="ExternalInput")
    W_out = nc.dram_tensor("W_out", [D, D], f32, kind="ExternalInput")
    b_out = nc.dram_tensor("b_out", [D], f32, kind="ExternalInput")
    out = nc.dram_tensor("out", [S, D], f32, kind="ExternalOutput")

    w_in_kp = W_in.rearrange("(ko p) n -> p ko n", p=P)  # [128, 8, 3072]
    w_out_kp = W_out.rearrange("(ko p) n -> p ko n", p=P)  # [128, 8, 1024]

    with TileContext(nc) as tc:
        const = tc.alloc_tile_pool(name="const", bufs=1)
        # PSUM: mps 2x[128,512] (2 banks) shared by A/B/C/E, dwide
        # 2x[128,1024] (4 banks) for scores, pvp 2x[128,512] (2 banks).
        mps = tc.alloc_tile_pool(name="mps", bufs=2, space="PSUM")
        dwide = tc.alloc_tile_pool(name="dwide", bufs=2, space="PSUM")
        pvp = tc.alloc_tile_pool(name="pvp", bufs=2, space="PSUM")

        identity = const.tile([P, P], bf16)
        make_identity(nc, identity[:])
        bqk = const.tile([P, NTQK], f32)
        nc.sync.dma_start(bqk[:], b_in[0 : 2 * D].rearrange("(o p) -> p o", p=P))
        bv_bc = const.tile([P, D], f32)
        bout_bc = const.tile([P, D], f32)
        ones4 = const.tile([P, ST, H, 1], f32)
        nc.vector.memset(ones4[:], 1.0)

        # ---------------- resident tensors ----------------
        qkT_pool = tc.alloc_tile_pool(name="qkT", bufs=1)
        qkT = qkT_pool.tile([P, NTQK, S], bf16)  # 4 MB
        vaug_pool = tc.alloc_tile_pool(name="vaug", bufs=1)
        v_aug = vaug_pool.tile([P, ST, H, DK + 1], bf16)  # 2.1 MB
        nc.vector.tensor_copy(v_aug[:, :, :, DK : DK + 1], ones4[:])
        xT_pool = tc.alloc_tile_pool(name="xT", bufs=1)
        xT = xT_pool.tile([P, DT, S], bf16)  # 2 MB
        attnT_pool = tc.alloc_tile_pool(name="attnT", bufs=1)
        attnT_lo = attnT_pool.tile([P, DT // 2, S], bf16)  # pairs 0-3
        attnT_hi = attnT_pool.tile([P, DT // 2, S], bf16)  # pairs 4-7
        # partial output projection over pairs 0-3 (+b_out), built during
        # pairs 4-7 so the E tail only covers the other half; parked in DRAM
        # scratch (DMA is idle there) to save SBUF
        y_part = nc.dram_tensor("y_part_scratch", [ST, P, D], bf16, kind="Internal")
        woutb_pool = tc.alloc_tile_pool(name="woutb", bufs=1)
        wout_bf = woutb_pool.tile([P, DT, D], bf16)  # 2 MB
        wv_pool = tc.alloc_tile_pool(name="wv", bufs=1)
        wv_tile = wv_pool.tile([P, DT, D], bf16)  # 2 MB

        pa = tc.alloc_tile_pool(name="phaseA", bufs=1)
        bv_row = pa.tile([1, D], f32)
        nc.sync.dma_start(bv_row[:], b_in[None, 2 * D : 3 * D])
        nc.gpsimd.partition_broadcast(bv_bc[:], bv_row[:])
        bout_row = pa.tile([1, D], f32)
        nc.sync.dma_start(bout_row[:], b_out[None, :])
        nc.gpsimd.partition_broadcast(bout_bc[:], bout_row[:])

        wqk = tc.alloc_tile_pool(name="wqk", bufs=3)

        def emit_w_dma(nt):
            w_stage = wqk.tile([P, DT, P], f32, tag="ws")
            nc.sync.dma_start(w_stage[:], w_in_kp[:, :, nt * P : (nt + 1) * P])
            return w_stage

        # ---------------- phase A: X^T (PE transpose, cast to bf16) --------
        w_pref = {}
        with tc.tile_pool(name="xstage", bufs=3) as xstage:
            for si in range(ST):
                x_tile = xstage.tile([P, D], f32, tag="x")
                nc.sync.dma_start(x_tile[:], X[si * P : (si + 1) * P, :])
                if si == ST - 1:
                    # pair-0 QK weights queue right behind the X transfers
                    w_pref = {0: emit_w_dma(0), PAIRS: emit_w_dma(PAIRS)}
                xb = xstage.tile([P, D], bf16, tag="xb")
                nc.vector.tensor_copy(xb[:], x_tile[:])
                for half in range(2):
                    tp = mps.tile([P, 512], f32, tag="w", name="tp")
                    for j in range(4):
                        dj = half * 4 + j
                        nc.tensor.matmul(
                            tp[:, j * P : (j + 1) * P],
                            xb[:, dj * P : (dj + 1) * P],
                            identity[:],
                            start=True,
                            stop=True,
                        )
                    nc.scalar.copy(
                        xT[:, half * 4 : (half + 1) * 4, si * P : (si + 1) * P],
                        tp[:].rearrange("p (j q) -> p j q", q=P),
                    )

        # ---------------- per-pair B (QK projection) ----------------
        def emit_b(pr):
            for nt in (pr, PAIRS + pr):
                w_stage = w_pref.pop(nt, None)
                if w_stage is None:
                    w_stage = emit_w_dma(nt)
                w_tile = wqk.tile([P, DT, P], bf16, tag="w")
                nc.vector.tensor_copy(w_tile[:], w_stage[:])
                ps = [
                    mps.tile([P, 512], f32, tag="w", name=f"psb{sc}")
                    for sc in range(SC)
                ]
                for dk in range(DT):
                    for sc in range(SC):
                        nc.tensor.matmul(
                            ps[sc][:],
                            w_tile[:, dk, :],
                            xT[:, dk, sc * 512 : (sc + 1) * 512],
                            start=(dk == 0),
                            stop=(dk == DT - 1),
                        )
                for sc in range(SC):
                    nc.vector.tensor_scalar(
                        qkT[:, nt, sc * 512 : (sc + 1) * 512],
                        ps[sc][:],
                        bqk[:, nt : nt + 1],
                        None,
                        ADD,
                    )

        # ---------------- per-pair D (attention) ----------------
        def emit_scores(pr, sc, sk, expp):
            wide = dwide.tile([P, S], f32, tag="sps", name="sps")
            for hh in range(2):
                base = hh * DK
                nc.tensor.matmul(
                    wide[:, hh * 512 : (hh + 1) * 512],
                    qkT[base : base + DK, PAIRS + pr, sk * P : (sk + 1) * P],
                    qkT[base : base + DK, pr, sc * 512 : (sc + 1) * 512],
                    start=True,
                    stop=True,
                )
            ex = expp.tile([P, S], bf16, tag="ex")
            if sk in DVE_EXP_SKS:
                nc.vector.tensor_scalar(
                    ex[:].bitcast(i16), wide[:], SCH_A, SCH_B, MULT, ADD
                )
            else:
                nc.scalar.activation(
                    ex[:], wide[:], EXP, scale=1.0 / np.sqrt(DK)
                )
            return ex

        def emit_pv(pr, sk, ex, pv):
            for hh in range(2):
                h = 2 * pr + hh
                nc.tensor.matmul(
                    pv[hh][0 : DK + 1, :],
                    v_aug[:, sk, h, :],
                    ex[:, hh * 512 : (hh + 1) * 512],
                    start=(sk == 0),
                    stop=(sk == ST - 1),
                )

        def emit_norm(pr, sc, pv, expp, bcp, rrowp):
            # Decouple normalization from the PSUM critical path: one bf16
            # copy per head frees the pv bank (values + denominator row);
            # the reciprocal chain then runs off-path from SBUF.
            upv = [
                rrowp.tile([P, 512], bf16, tag=f"upv{hh}", name=f"upv{hh}")
                for hh in range(2)
            ]
            for hh in range(2):
                nc.vector.tensor_copy(upv[hh][0 : DK + 1, :], pv[hh][0 : DK + 1, :])
            drow = rrowp.tile([1, S], f32, tag="dr", name="drow")
            for hh in range(2):
                # custom-DVE ops mis-read nonzero base_partition on HW:
                # stage the partition-64 denominator rows at partition 0
                nc.scalar.copy(
                    drow[:, hh * 512 : (hh + 1) * 512],
                    upv[hh][DK : DK + 1, :],
                )
            nc.vector.reciprocal_approx_fast(drow[:], drow[:])
            # full-tile broadcast (sliced outputs break on HW)
            bc = bcp.tile([P, S], f32, tag="bc", name="bc")
            nc.gpsimd.partition_broadcast(bc[:], drow[:])
            attnT_half = attnT_lo if pr < DT // 2 else attnT_hi
            for hh in range(2):
                base = hh * DK
                nc.vector.tensor_tensor(
                    attnT_half[
                        base : base + DK, pr % (DT // 2),
                        sc * 512 : (sc + 1) * 512,
                    ],
                    upv[hh][0:DK, :],
                    bc[0:DK, hh * 512 : (hh + 1) * 512],
                    MULT,
                )

        def emit_d_block(pr, sc, expp, bcp, rrowp, defer_hook=None):
            pv_alloc = lambda: [
                pvp.tile([P, 512], f32, tag="pv", name=f"pv{i}")
                for i in range(2)
            ]
            if defer_hook is None:
                pv = pv_alloc()
                for sk in range(ST):
                    ex = emit_scores(pr, sc, sk, expp)
                    emit_pv(pr, sk, ex, pv)
            else:
                exs = [emit_scores(pr, sc, sk, expp) for sk in range(ST)]
                defer_hook()
                pv = pv_alloc()
                for sk in range(ST):
                    emit_pv(pr, sk, exs[sk], pv)
            emit_norm(pr, sc, pv, expp, bcp, rrowp)

        with (
            tc.tile_pool(name="wstage", bufs=2) as wstage,
            tc.tile_pool(name="expp", bufs=7) as expp,
            tc.tile_pool(name="bcp", bufs=2) as bcp,
            tc.tile_pool(name="rrow", bufs=2) as rrowp,
        ):

            # stage V weights right behind pair-0 QK on the DMA queue
            for ch in range(4):
                wv_stage = wstage.tile([P, DT, 256], f32, tag="wvs")
                nc.sync.dma_start(
                    wv_stage[:],
                    w_in_kp[:, :, 2 * D + ch * 256 : 2 * D + (ch + 1) * 256],
                )
                nc.vector.tensor_copy(
                    wv_tile[:, :, ch * 256 : (ch + 1) * 256], wv_stage[:]
                )
            # W_out staged load + bf16 cast (transfers overlap B/C/D)
            for ch in range(4):
                wo_stage = wstage.tile([P, DT, 256], f32, tag="wvs")
                nc.sync.dma_start(
                    wo_stage[:],
                    w_out_kp[:, :, ch * 256 : (ch + 1) * 256],
                )
                nc.vector.tensor_copy(
                    wout_bf[:, :, ch * 256 : (ch + 1) * 256], wo_stage[:]
                )

            # ---------------- phase C: V projection (split by ncx) -------
            def emit_c(ncx):
                for st in range(ST):
                    ps = mps.tile([P, 512], f32, tag="w", name="psc")
                    for dk in range(DT):
                        nc.tensor.matmul(
                            ps[:],
                            xT[:, dk, st * P : (st + 1) * P],
                            wv_tile[:, dk, ncx * 512 : (ncx + 1) * 512],
                            start=(dk == 0),
                            stop=(dk == DT - 1),
                        )
                    nc.vector.tensor_tensor(
                        v_aug[:, st, 8 * ncx : 8 * (ncx + 1), 0:DK],
                        ps[:].rearrange("p (h d) -> p h d", d=DK),
                        bv_bc[:, ncx * 512 : (ncx + 1) * 512].rearrange(
                            "p (h d) -> p h d", d=DK
                        ),
                        ADD,
                    )

            emit_b(0)
            # D(0,0) scores+exps run during C; its PVs come after C
            emit_d_block(
                0, 0, expp, bcp, rrowp,
                defer_hook=lambda: (emit_c(0), emit_c(1)),
            )
            emit_d_block(0, 1, expp, bcp, rrowp)

            # first-half output projection, spread across pairs 4-7
            def emit_e1(stp):
                for st in (2 * stp, 2 * stp + 1):
                    ps = [
                        mps.tile([P, 512], f32, tag="w", name=f"ps1{ncx}")
                        for ncx in range(SC)
                    ]
                    for dk in range(DT // 2):
                        for ncx in range(SC):
                            nc.tensor.matmul(
                                ps[ncx][:],
                                attnT_lo[:, dk, st * P : (st + 1) * P],
                                wout_bf[:, dk, ncx * 512 : (ncx + 1) * 512],
                                start=(dk == 0),
                                stop=(dk == DT // 2 - 1),
                            )
                    y1 = rrowp.tile([P, D], bf16, tag="y1", name="y1")
                    for ncx in range(SC):
                        nc.vector.tensor_tensor(
                            y1[:, ncx * 512 : (ncx + 1) * 512],
                            ps[ncx][:],
                            bout_bc[:, ncx * 512 : (ncx + 1) * 512],
                            ADD,
                        )
                    nc.sync.dma_start(y_part[st], y1[:])

            # ---------------- remaining pairs ----------------
            for pr in range(1, PAIRS):
                emit_b(pr)
                if pr >= 4:
                    emit_e1(pr - 4)
                for sc in range(SC):
                    emit_d_block(pr, sc, expp, bcp, rrowp)


        # ------------- phase E: second-half projection + y_part -------------
        with tc.tile_pool(name="ypool", bufs=2) as ypool:
            for st in range(ST):
                y1b = ypool.tile([P, D], bf16, tag="y1b")
                nc.sync.dma_start(y1b[:], y_part[st])
                ps = [
                    mps.tile([P, 512], f32, tag="w", name=f"pse{ncx}")
                    for ncx in range(SC)
                ]
                for dk in range(DT // 2):
                    for ncx in range(SC):
                        nc.tensor.matmul(
                            ps[ncx][:],
                            attnT_hi[:, dk, st * P : (st + 1) * P],
                            wout_bf[
                                :, DT // 2 + dk, ncx * 512 : (ncx + 1) * 512
                            ],
                            start=(dk == 0),
                            stop=(dk == DT // 2 - 1),
                        )
                y = ypool.tile([P, D], f32, tag="y")
                for ncx in range(SC):
                    nc.vector.tensor_tensor(
                        y[:, ncx * 512 : (ncx + 1) * 512],
                        ps[ncx][:],
                        y1b[:, ncx * 512 : (ncx + 1) * 512],
                        ADD,
                    )
                nc.sync.dma_start(out[st * P : (st + 1) * P, :], y[:])

        for pool in (
            wqk,
            pa,
            wv_pool,
            woutb_pool,
            attnT_pool,
            xT_pool,
            vaug_pool,
            qkT_pool,
            pvp,
            dwide,
            mps,
            const,
        ):
            pool.release()

    nc.finalize()
    return nc


_NC_CACHE = {}


def get_nc():
    if "nc" not in _NC_CACHE:
        _NC_CACHE["nc"] = build_nc()
    return _NC_CACHE["nc"]


def kernel(X, W_in, b_in, W_out, b_out):
    X = np.ascontiguousarray(np.asarray(X, dtype=np.float32))
    W_in = np.ascontiguousarray(np.asarray(W_in, dtype=np.float32))
    b_in = np.ascontiguousarray(np.asarray(b_in, dtype=np.float32))
    W_out = np.ascontiguousarray(np.asarray(W_out, dtype=np.float32))
    b_out = np.ascontiguousarray(np.asarray(b_out, dtype=np.float32))

    nc = get_nc()
    in_maps = [
        {"X": X[i], "W_in": W_in, "b_in": b_in, "W_out": W_out, "b_out": b_out}
        for i in range(B)
    ]
    res = run_bass_kernel_spmd(nc, in_maps, core_ids=list(range(B)))
    return np.stack([res.results[i]["out"] for i in range(B)], axis=0)


# revision 32
# speedup vs baseline: 1.0367x; 1.0301x over previous
"""Multi-head attention (B=8, S=1024, D=1024, H=16) on 8 TRN2 NeuronCores.

Sharding: pure data parallel - batch element b on core b. Weights are
broadcast to every core. No collectives.

Per-core pipeline (X: [S, D] for one batch element), restructured so the
softmax exp (the ACT-engine bottleneck) overlaps PE matmul work:

  A. X^T via PE transposes (bf16 matmul against identity), 4 blocks per
     PSUM bank, one wide ACT copy per 4 blocks.
  B(pr). Q^T/K^T projection for head pair pr only (nt = pr and 8+pr),
     dk-outer / sc-inner loop so each LDWEIGHTS feeds 2 matmuls; bias
     (+psum->bf16 copy) on DVE to keep ACT free for exp.
  C. V = X @ W_v (natural layout) into V_aug[sk, head, 65] with a ones
     column (col 64) that makes the PV matmul accumulate the softmax
     denominator in row 64.
  D(pr). attention for the pair: scores^T = K_h^T.T @ Q_h^T into a wide
     [128,1024] PSUM tile (two heads row-split, concurrent on the PE),
     exp split 6:2 between ACT (exact, scale=1/8) and DVE (Schraudolph
     int16 bit-trick writing bf16 bit patterns), PV with the ones row.
     Normalize is decoupled from the PSUM path: one bf16 copy per head
     frees the pv bank, then reciprocal_approx_fast (from a partition-0
     staging row - custom-DVE ops mis-read nonzero base partitions on
     HW) + GpSimd partition-broadcast + DVE multiply -> attnT bf16.
  E. Y = attn_out @ W_out + b_out in bf16.

Emission order A; B(0); D(0,0){scores; C as defer hook; PV}; D(0,1);
B(1); D(1); ... B(7); D(7); E lets the Tile scheduler run B(pr+1)
matmuls in the PE gaps of D(pr) while ACT/DVE stream the exps, and run
D(0)'s scores/exps during C. PSUM: 2x[128,512] misc + 2x[128,1024]
scores + 2x[128,512] pv = 8 banks.
"""

import sys

sys.path.insert(0, "/opt/trn_rl_repo")

import numpy as np

import concourse.bacc as bacc
import concourse.mybir as mybir
from concourse.bass_utils import run_bass_kernel_spmd
from concourse.masks import make_identity
from concourse.tile import TileContext

B = 8
S = 1024
D = 1024
H = 16
DK = D // H  # 64
P = 128
ST = S // P   # 8 s-tiles
DT = D // P   # 8 d-tiles
NTQK = 2 * D // P  # 16 n-tiles for the Q|K part
PAIRS = H // 2     # 8 head pairs
SC = S // 512      # 2 chunks of 512 (matmul free-dim limit)

f32 = mybir.dt.float32
bf16 = mybir.dt.bfloat16
i16 = mybir.dt.int16
EXP = mybir.ActivationFunctionType.Exp
MULT = mybir.AluOpType.mult
ADD = mybir.AluOpType.add

# Schraudolph exp on DVE: bf16 bits of exp(s/8) = int16((a/(8*65536))*s + b)
# with a = 2^23/ln2, b = (127*2^23 - C)/65536, C = 366393 (min max-rel-err).
SCH_A = float(2.0**23 / np.log(2.0) / (8.0 * 65536.0))
SCH_B = float((127 * 2.0**23 - 366393.0) / 65536.0)
# which sk-tiles of each (pair, sc) block use the DVE exp (rest use ACT)
DVE_EXP_SKS = frozenset({3, 7})


def build_nc():
    nc = bacc.Bacc()
    X = nc.dram_tensor("X", [S, D], f32, kind="ExternalInput")
    W_in = nc.dram_tensor("W_in", [D, 3 * D], f32, kind="ExternalInput")
    b_in = nc.dram_tensor("b_in", [3 * D], f32, kind="ExternalInput")
    W_out = nc.dram_tensor("W_out", [D, D], f32, kind="ExternalInput")
    b_out = nc.dram_tensor("b_out", [D], f32, kind="ExternalInput")
    out = nc.dram_tensor("out", [S, D], f32, kind="ExternalOutput")

    w_in_kp = W_in.rearrange("(ko p) n -> p ko n", p=P)  # [128, 8, 3072]
    w_out_kp = W_out.rearrange("(ko p) n -> p ko n", p=P)  # [128, 8, 1024]

    with TileContext(nc) as tc:
        const = tc.alloc_tile_pool(name="const", bufs=1)
        # PSUM: mps 2x[128,512] (2 banks) shared by A/B/C/E, dwide
        # 2x[128,1024] (4 banks) for scores, pvp 2x[128,512] (2 banks).
        mps = tc.alloc_tile_pool(name="mps", bufs=2, space="PSUM")
        dwide = tc.alloc_tile_pool(name="dwide", bufs=2, space="PSUM")
        pvp = tc.alloc_tile_pool(name="pvp", bufs=2, space="PSUM")

        identity = const.tile([P, P], bf16)
        make_identity(nc, identity[:])
        bqk = const.tile([P, NTQK], f32)
        nc.sync.dma_start(bqk[:], b_in[0 : 2 * D].rearrange("(o p) -> p o", p=P))
        bv_bc = const.tile([P, D], f32)
        bout_bc = const.tile([P, D], f32)
        ones4 = const.tile([P, ST, H, 1], f32)
        nc.vector.memset(ones4[:], 1.0)

        # ---------------- resident tensors ----------------
        qkT_pool = tc.alloc_tile_pool(name="qkT", bufs=1)
        qkT = qkT_pool.tile([P, NTQK, S], bf16)  # 4 MB
        vaug_pool = tc.alloc_tile_pool(name="vaug", bufs=1)
        v_aug = vaug_pool.tile([P, ST, H, DK + 1], bf16)  # 2.1 MB
        nc.vector.tensor_copy(v_aug[:, :, :, DK : DK + 1], ones4[:])
        xT_pool = tc.alloc_tile_pool(name="xT", bufs=1)
        xT = xT_pool.tile([P, DT, S], bf16)  # 2 MB
        attnT_pool = tc.alloc_tile_pool(name="attnT", bufs=1)
        attnT = attnT_pool.tile([P, DT, S], bf16)  # 2 MB
        woutb_pool = tc.alloc_tile_pool(name="woutb", bufs=1)
        wout_bf = woutb_pool.tile([P, DT, D], bf16)  # 2 MB
        wv_pool = tc.alloc_tile_pool(name="wv", bufs=1)
        wv_tile = wv_pool.tile([P, DT, D], bf16)  # 2 MB

        pa = tc.alloc_tile_pool(name="phaseA", bufs=1)
        bv_row = pa.tile([1, D], f32)
        nc.sync.dma_start(bv_row[:], b_in[None, 2 * D : 3 * D])
        nc.gpsimd.partition_broadcast(bv_bc[:], bv_row[:])
        bout_row = pa.tile([1, D], f32)
        nc.sync.dma_start(bout_row[:], b_out[None, :])
        nc.gpsimd.partition_broadcast(bout_bc[:], bout_row[:])

        wqk = tc.alloc_tile_pool(name="wqk", bufs=3)

        def emit_w_dma(nt):
            w_stage = wqk.tile([P, DT, P], f32, tag="ws")
            nc.sync.dma_start(w_stage[:], w_in_kp[:, :, nt * P : (nt + 1) * P])
            return w_stage

        # ---------------- phase A: X^T (PE transpose, cast to bf16) --------
        w_pref = {}
        with tc.tile_pool(name="xstage", bufs=3) as xstage:
            for si in range(ST):
                x_tile = xstage.tile([P, D], f32, tag="x")
                nc.sync.dma_start(x_tile[:], X[si * P : (si + 1) * P, :])
                if si == 0:
                    # prefetch pair-0 QK weights right after the first X tile
                    w_pref = {0: emit_w_dma(0), PAIRS: emit_w_dma(PAIRS)}
                xb = xstage.tile([P, D], bf16, tag="xb")
                nc.vector.tensor_copy(xb[:], x_tile[:])
                for half in range(2):
                    tp = mps.tile([P, 512], f32, tag="w", name="tp")
                    for j in range(4):
                        dj = half * 4 + j
                        nc.tensor.matmul(
                            tp[:, j * P : (j + 1) * P],
                            xb[:, dj * P : (dj + 1) * P],
                            identity[:],
                            start=True,
                            stop=True,
                        )
                    nc.scalar.copy(
                        xT[:, half * 4 : (half + 1) * 4, si * P : (si + 1) * P],
                        tp[:].rearrange("p (j q) -> p j q", q=P),
                    )

        # ---------------- per-pair B (QK projection) ----------------
        def emit_b(pr):
            for nt in (pr, PAIRS + pr):
                w_stage = w_pref.pop(nt, None)
                if w_stage is None:
                    w_stage = emit_w_dma(nt)
                w_tile = wqk.tile([P, DT, P], bf16, tag="w")
                nc.vector.tensor_copy(w_tile[:], w_stage[:])
                ps = [
                    mps.tile([P, 512], f32, tag="w", name=f"psb{sc}")
                    for sc in range(SC)
                ]
                for dk in range(DT):
                    for sc in range(SC):
                        nc.tensor.matmul(
                            ps[sc][:],
                            w_tile[:, dk, :],
                            xT[:, dk, sc * 512 : (sc + 1) * 512],
                            start=(dk == 0),
                            stop=(dk == DT - 1),
                        )
                for sc in range(SC):
                    nc.vector.tensor_scalar(
                        qkT[:, nt, sc * 512 : (sc + 1) * 512],
                        ps[sc][:],
                        bqk[:, nt : nt + 1],
                        None,
                        ADD,
                    )

        # ---------------- per-pair D (attention) ----------------
        def emit_scores(pr, sc, sk, expp):
            wide = dwide.tile([P, S], f32, tag="sps", name="sps")
            for hh in range(2):
                base = hh * DK
                nc.tensor.matmul(
                    wide[:, hh * 512 : (hh + 1) * 512],
                    qkT[base : base + DK, PAIRS + pr, sk * P : (sk + 1) * P],
                    qkT[base : base + DK, pr, sc * 512 : (sc + 1) * 512],
                    start=True,
                    stop=True,
                )
            ex = expp.tile([P, S], bf16, tag="ex")
            if sk in DVE_EXP_SKS:
                nc.vector.tensor_scalar(
                    ex[:].bitcast(i16), wide[:], SCH_A, SCH_B, MULT, ADD
                )
            else:
                nc.scalar.activation(
                    ex[:], wide[:], EXP, scale=1.0 / np.sqrt(DK)
                )
            return ex

        def emit_pv(pr, sk, ex, pv):
            for hh in range(2):
                h = 2 * pr + hh
                nc.tensor.matmul(
                    pv[hh][0 : DK + 1, :],
                    v_aug[:, sk, h, :],
                    ex[:, hh * 512 : (hh + 1) * 512],
                    start=(sk == 0),
                    stop=(sk == ST - 1),
                )

        def emit_norm(pr, sc, pv, expp, bcp, rrowp):
            # Decouple normalization from the PSUM critical path: one bf16
            # copy per head frees the pv bank (values + denominator row);
            # the reciprocal chain then runs off-path from SBUF.
            upv = [
                rrowp.tile([P, 512], bf16, tag=f"upv{hh}", name=f"upv{hh}")
                for hh in range(2)
            ]
            for hh in range(2):
                nc.vector.tensor_copy(upv[hh][0 : DK + 1, :], pv[hh][0 : DK + 1, :])
            drow = rrowp.tile([1, S], f32, tag="dr", name="drow")
            for hh in range(2):
                # custom-DVE ops mis-read nonzero base_partition on HW:
                # stage the partition-64 denominator rows at partition 0
                nc.scalar.copy(
                    drow[:, hh * 512 : (hh + 1) * 512],
                    upv[hh][DK : DK + 1, :],
                )
            nc.vector.reciprocal_approx_fast(drow[:], drow[:])
            # full-tile broadcast (sliced outputs break on HW)
            bc = bcp.tile([P, S], f32, tag="bc", name="bc")
            nc.gpsimd.partition_broadcast(bc[:], drow[:])
            for hh in range(2):
                base = hh * DK
                nc.vector.tensor_tensor(
                    attnT[base : base + DK, pr, sc * 512 : (sc + 1) * 512],
                    upv[hh][0:DK, :],
                    bc[0:DK, hh * 512 : (hh + 1) * 512],
                    MULT,
                )

        def emit_d_block(pr, sc, expp, bcp, rrowp, defer_hook=None):
            pv_alloc = lambda: [
                pvp.tile([P, 512], f32, tag="pv", name=f"pv{i}")
                for i in range(2)
            ]
            if defer_hook is None:
                pv = pv_alloc()
                for sk in range(ST):
                    ex = emit_scores(pr, sc, sk, expp)
                    emit_pv(pr, sk, ex, pv)
            else:
                exs = [emit_scores(pr, sc, sk, expp) for sk in range(ST)]
                defer_hook()
                pv = pv_alloc()
                for sk in range(ST):
                    emit_pv(pr, sk, exs[sk], pv)
            emit_norm(pr, sc, pv, expp, bcp, rrowp)

        with (
            tc.tile_pool(name="wstage", bufs=2) as wstage,
            tc.tile_pool(name="expp", bufs=7) as expp,
            tc.tile_pool(name="bcp", bufs=2) as bcp,
            tc.tile_pool(name="rrow", bufs=2) as rrowp,
        ):

            # ---------------- phase C: V projection (split by ncx) -------
            def emit_c(ncx):
                for ch in (2 * ncx, 2 * ncx + 1):
                    wv_stage = wstage.tile([P, DT, 256], f32, tag="wvs")
                    nc.sync.dma_start(
                        wv_stage[:],
                        w_in_kp[
                            :, :, 2 * D + ch * 256 : 2 * D + (ch + 1) * 256
                        ],
                    )
                    nc.vector.tensor_copy(
                        wv_tile[:, :, ch * 256 : (ch + 1) * 256], wv_stage[:]
                    )
                for st in range(ST):
                    ps = mps.tile([P, 512], f32, tag="w", name="psc")
                    for dk in range(DT):
                        nc.tensor.matmul(
                            ps[:],
                            xT[:, dk, st * P : (st + 1) * P],
                            wv_tile[:, dk, ncx * 512 : (ncx + 1) * 512],
                            start=(dk == 0),
                            stop=(dk == DT - 1),
                        )
                    nc.vector.tensor_tensor(
                        v_aug[:, st, 8 * ncx : 8 * (ncx + 1), 0:DK],
                        ps[:].rearrange("p (h d) -> p h d", d=DK),
                        bv_bc[:, ncx * 512 : (ncx + 1) * 512].rearrange(
                            "p (h d) -> p h d", d=DK
                        ),
                        ADD,
                    )

            emit_b(0)
            # D(0,0) scores+exps run during C; its PVs come after C
            emit_d_block(
                0, 0, expp, bcp, rrowp,
                defer_hook=lambda: (emit_c(0), emit_c(1)),
            )
            emit_d_block(0, 1, expp, bcp, rrowp)

            # ---------------- remaining pairs ----------------
            for pr in range(1, PAIRS):
                emit_b(pr)
                for sc in range(SC):
                    emit_d_block(pr, sc, expp, bcp, rrowp)

            # W_out staged load + bf16 cast (overlaps the pair loop)
            for ch in range(4):
                wo_stage = wstage.tile([P, DT, 256], f32, tag="wvs")
                nc.sync.dma_start(
                    wo_stage[:],
                    w_out_kp[:, :, ch * 256 : (ch + 1) * 256],
                )
                nc.vector.tensor_copy(
                    wout_bf[:, :, ch * 256 : (ch + 1) * 256], wo_stage[:]
                )

        # ---------------- phase E: output projection (bf16) ----------------
        with tc.tile_pool(name="ypool", bufs=2) as ypool:
            for st in range(ST):
                ps = [
                    mps.tile([P, 512], f32, tag="w", name=f"pse{ncx}")
                    for ncx in range(SC)
                ]
                for dk in range(DT):
                    for ncx in range(SC):
                        nc.tensor.matmul(
                            ps[ncx][:],
                            attnT[:, dk, st * P : (st + 1) * P],
                            wout_bf[:, dk, ncx * 512 : (ncx + 1) * 512],
                            start=(dk == 0),
                            stop=(dk == DT - 1),
                        )
                y = ypool.tile([P, D], f32, tag="y")
                for ncx in range(SC):
                    nc.vector.tensor_tensor(
                        y[:, ncx * 512 : (ncx + 1) * 512],
                        ps[ncx][:],
                        bout_bc[:, ncx * 512 : (ncx + 1) * 512],
                        ADD,
                    )
                nc.sync.dma_start(out[st * P : (st + 1) * P, :], y[:])

        for pool in (
            wqk,
            pa,
            wv_pool,
            woutb_pool,
            attnT_pool,
            xT_pool,
            vaug_pool,
            qkT_pool,
            pvp,
            dwide,
            mps,
            const,
        ):
            pool.release()

    nc.finalize()
    return nc


_NC_CACHE = {}


def get_nc():
    if "nc" not in _NC_CACHE:
        _NC_CACHE["nc"] = build_nc()
    return _NC_CACHE["nc"]


def kernel(X, W_in, b_in, W_out, b_out):
    X = np.ascontiguousarray(np.asarray(X, dtype=np.float32))
    W_in = np.ascontiguousarray(np.asarray(W_in, dtype=np.float32))
    b_in = np.ascontiguousarray(np.asarray(b_in, dtype=np.float32))
    W_out = np.ascontiguousarray(np.asarray(W_out, dtype=np.float32))
    b_out = np.ascontiguousarray(np.asarray(b_out, dtype=np.float32))

    nc = get_nc()
    in_maps = [
        {"X": X[i], "W_in": W_in, "b_in": b_in, "W_out": W_out, "b_out": b_out}
        for i in range(B)
    ]
    res = run_bass_kernel_spmd(nc, in_maps, core_ids=list(range(B)))
    return np.stack([res.results[i]["out"] for i in range(B)], axis=0)


# revision 34
# speedup vs baseline: 1.0388x; 1.0021x over previous
"""Multi-head attention (B=8, S=1024, D=1024, H=16) on 8 TRN2 NeuronCores.

Sharding: pure data parallel - batch element b on core b. Weights are
broadcast to every core. No collectives.

Per-core pipeline (X: [S, D] for one batch element), restructured so the
softmax exp (the ACT-engine bottleneck) overlaps PE matmul work:

  A. X^T via PE transposes (bf16 matmul against identity), 4 blocks per
     PSUM bank, one wide ACT copy per 4 blocks.
  B(pr). Q^T/K^T projection for head pair pr only (nt = pr and 8+pr),
     dk-outer / sc-inner loop so each LDWEIGHTS feeds 2 matmuls; bias
     (+psum->bf16 copy) on DVE to keep ACT free for exp.
  C. V = X @ W_v (natural layout) into V_aug[sk, head, 65] with a ones
     column (col 64) that makes the PV matmul accumulate the softmax
     denominator in row 64.
  D(pr). attention for the pair: scores^T = K_h^T.T @ Q_h^T into a wide
     [128,1024] PSUM tile (two heads row-split, concurrent on the PE),
     exp split 6:2 between ACT (exact, scale=1/8) and DVE (Schraudolph
     int16 bit-trick writing bf16 bit patterns), PV with the ones row.
     Normalize is decoupled from the PSUM path: one bf16 copy per head
     frees the pv bank, then reciprocal_approx_fast (from a partition-0
     staging row - custom-DVE ops mis-read nonzero base partitions on
     HW) + GpSimd partition-broadcast + DVE multiply -> attnT bf16.
  E. Y = attn_out @ W_out + b_out in bf16.

Emission order A; B(0); D(0,0){scores; C as defer hook; PV}; D(0,1);
B(1); D(1); ... B(7); D(7); E lets the Tile scheduler run B(pr+1)
matmuls in the PE gaps of D(pr) while ACT/DVE stream the exps, and run
D(0)'s scores/exps during C. PSUM: 2x[128,512] misc + 2x[128,1024]
scores + 2x[128,512] pv = 8 banks.
"""

import sys

sys.path.insert(0, "/opt/trn_rl_repo")

import numpy as np

import concourse.bacc as bacc
import concourse.mybir as mybir
from concourse.bass_utils import run_bass_kernel_spmd
from concourse.masks import make_identity
from concourse.tile import TileContext

B = 8
S = 1024
D = 1024
H = 16
DK = D // H  # 64
P = 128
ST = S // P   # 8 s-tiles
DT = D // P   # 8 d-tiles
NTQK = 2 * D // P  # 16 n-tiles for the Q|K part
PAIRS = H // 2     # 8 head pairs
SC = S // 512      # 2 chunks of 512 (matmul free-dim limit)

f32 = mybir.dt.float32
bf16 = mybir.dt.bfloat16
i16 = mybir.dt.int16
EXP = mybir.ActivationFunctionType.Exp
MULT = mybir.AluOpType.mult
ADD = mybir.AluOpType.add

# Schraudolph exp on DVE: bf16 bits of exp(s/8) = int16((a/(8*65536))*s + b)
# with a = 2^23/ln2, b = (127*2^23 - C)/65536, C = 366393 (min max-rel-err).
SCH_A = float(2.0**23 / np.log(2.0) / (8.0 * 65536.0))
SCH_B = float((127 * 2.0**23 - 366393.0) / 65536.0)
# which sk-tiles of each (pair, sc) block use the DVE exp (rest use ACT)
DVE_EXP_SKS = frozenset({3, 7})


def build_nc():
    nc = bacc.Bacc()
    X = nc.dram_tensor("X", [S, D], f32, kind="ExternalInput")
    W_in = nc.dram_tensor("W_in", [D, 3 * D], f32, kind="ExternalInput")
    b_in = nc.dram_tensor("b_in", [3 * D], f32, kind="ExternalInput")
    W_out = nc.dram_tensor("W_out", [D, D], f32, kind="ExternalInput")
    b_out = nc.dram_tensor("b_out", [D], f32, kind="ExternalInput")
    out = nc.dram_tensor("out", [S, D], f32, kind="ExternalOutput")

    w_in_kp = W_in.rearrange("(ko p) n -> p ko n", p=P)  # [128, 8, 3072]
    w_out_kp = W_out.rearrange("(ko p) n -> p ko n", p=P)  # [128, 8, 1024]

    with TileContext(nc) as tc:
        const = tc.alloc_tile_pool(name="const", bufs=1)
        # PSUM: mps 2x[128,512] (2 banks) shared by A/B/C/E, dwide
        # 2x[128,1024] (4 banks) for scores, pvp 2x[128,512] (2 banks).
        mps = tc.alloc_tile_pool(name="mps", bufs=2, space="PSUM")
        dwide = tc.alloc_tile_pool(name="dwide", bufs=2, space="PSUM")
        pvp = tc.alloc_tile_pool(name="pvp", bufs=2, space="PSUM")

        identity = const.tile([P, P], bf16)
        make_identity(nc, identity[:])
        bqk = const.tile([P, NTQK], f32)
        nc.sync.dma_start(bqk[:], b_in[0 : 2 * D].rearrange("(o p) -> p o", p=P))
        bv_bc = const.tile([P, D], f32)
        bout_bc = const.tile([P, D], f32)
        ones4 = const.tile([P, ST, H, 1], f32)
        nc.vector.memset(ones4[:], 1.0)

        # ---------------- resident tensors ----------------
        qkT_pool = tc.alloc_tile_pool(name="qkT", bufs=1)
        qkT = qkT_pool.tile([P, NTQK, S], bf16)  # 4 MB
        vaug_pool = tc.alloc_tile_pool(name="vaug", bufs=1)
        v_aug = vaug_pool.tile([P, ST, H, DK + 1], bf16)  # 2.1 MB
        nc.vector.tensor_copy(v_aug[:, :, :, DK : DK + 1], ones4[:])
        xT_pool = tc.alloc_tile_pool(name="xT", bufs=1)
        xT = xT_pool.tile([P, DT, S], bf16)  # 2 MB
        attnT_pool = tc.alloc_tile_pool(name="attnT", bufs=1)
        attnT = attnT_pool.tile([P, DT, S], bf16)  # 2 MB
        woutb_pool = tc.alloc_tile_pool(name="woutb", bufs=1)
        wout_bf = woutb_pool.tile([P, DT, D], bf16)  # 2 MB
        wv_pool = tc.alloc_tile_pool(name="wv", bufs=1)
        wv_tile = wv_pool.tile([P, DT, D], bf16)  # 2 MB

        pa = tc.alloc_tile_pool(name="phaseA", bufs=1)
        bv_row = pa.tile([1, D], f32)
        nc.sync.dma_start(bv_row[:], b_in[None, 2 * D : 3 * D])
        nc.gpsimd.partition_broadcast(bv_bc[:], bv_row[:])
        bout_row = pa.tile([1, D], f32)
        nc.sync.dma_start(bout_row[:], b_out[None, :])
        nc.gpsimd.partition_broadcast(bout_bc[:], bout_row[:])

        wqk = tc.alloc_tile_pool(name="wqk", bufs=3)

        def emit_w_dma(nt):
            w_stage = wqk.tile([P, DT, P], f32, tag="ws")
            nc.sync.dma_start(w_stage[:], w_in_kp[:, :, nt * P : (nt + 1) * P])
            return w_stage

        # ---------------- phase A: X^T (PE transpose, cast to bf16) --------
        w_pref = {}
        with tc.tile_pool(name="xstage", bufs=3) as xstage:
            for si in range(ST):
                x_tile = xstage.tile([P, D], f32, tag="x")
                nc.sync.dma_start(x_tile[:], X[si * P : (si + 1) * P, :])
                if si == 0:
                    # prefetch pair-0 QK weights right after the first X tile
                    w_pref = {0: emit_w_dma(0), PAIRS: emit_w_dma(PAIRS)}
                xb = xstage.tile([P, D], bf16, tag="xb")
                nc.vector.tensor_copy(xb[:], x_tile[:])
                for half in range(2):
                    tp = mps.tile([P, 512], f32, tag="w", name="tp")
                    for j in range(4):
                        dj = half * 4 + j
                        nc.tensor.matmul(
                            tp[:, j * P : (j + 1) * P],
                            xb[:, dj * P : (dj + 1) * P],
                            identity[:],
                            start=True,
                            stop=True,
                        )
                    nc.scalar.copy(
                        xT[:, half * 4 : (half + 1) * 4, si * P : (si + 1) * P],
                        tp[:].rearrange("p (j q) -> p j q", q=P),
                    )

        # ---------------- per-pair B (QK projection) ----------------
        def emit_b(pr):
            for nt in (pr, PAIRS + pr):
                w_stage = w_pref.pop(nt, None)
                if w_stage is None:
                    w_stage = emit_w_dma(nt)
                w_tile = wqk.tile([P, DT, P], bf16, tag="w")
                nc.vector.tensor_copy(w_tile[:], w_stage[:])
                ps = [
                    mps.tile([P, 512], f32, tag="w", name=f"psb{sc}")
                    for sc in range(SC)
                ]
                for dk in range(DT):
                    for sc in range(SC):
                        nc.tensor.matmul(
                            ps[sc][:],
                            w_tile[:, dk, :],
                            xT[:, dk, sc * 512 : (sc + 1) * 512],
                            start=(dk == 0),
                            stop=(dk == DT - 1),
                        )
                for sc in range(SC):
                    nc.vector.tensor_scalar(
                        qkT[:, nt, sc * 512 : (sc + 1) * 512],
                        ps[sc][:],
                        bqk[:, nt : nt + 1],
                        None,
                        ADD,
                    )

        # ---------------- per-pair D (attention) ----------------
        def emit_scores(pr, sc, sk, expp):
            wide = dwide.tile([P, S], f32, tag="sps", name="sps")
            for hh in range(2):
                base = hh * DK
                nc.tensor.matmul(
                    wide[:, hh * 512 : (hh + 1) * 512],
                    qkT[base : base + DK, PAIRS + pr, sk * P : (sk + 1) * P],
                    qkT[base : base + DK, pr, sc * 512 : (sc + 1) * 512],
                    start=True,
                    stop=True,
                )
            ex = expp.tile([P, S], bf16, tag="ex")
            if sk in DVE_EXP_SKS:
                nc.vector.tensor_scalar(
                    ex[:].bitcast(i16), wide[:], SCH_A, SCH_B, MULT, ADD
                )
            else:
                nc.scalar.activation(
                    ex[:], wide[:], EXP, scale=1.0 / np.sqrt(DK)
                )
            return ex

        def emit_pv(pr, sk, ex, pv):
            for hh in range(2):
                h = 2 * pr + hh
                nc.tensor.matmul(
                    pv[hh][0 : DK + 1, :],
                    v_aug[:, sk, h, :],
                    ex[:, hh * 512 : (hh + 1) * 512],
                    start=(sk == 0),
                    stop=(sk == ST - 1),
                )

        def emit_norm(pr, sc, pv, expp, bcp, rrowp):
            # Decouple normalization from the PSUM critical path: one bf16
            # copy per head frees the pv bank (values + denominator row);
            # the reciprocal chain then runs off-path from SBUF.
            upv = [
                rrowp.tile([P, 512], bf16, tag=f"upv{hh}", name=f"upv{hh}")
                for hh in range(2)
            ]
            for hh in range(2):
                nc.vector.tensor_copy(upv[hh][0 : DK + 1, :], pv[hh][0 : DK + 1, :])
            drow = rrowp.tile([1, S], f32, tag="dr", name="drow")
            for hh in range(2):
                # custom-DVE ops mis-read nonzero base_partition on HW:
                # stage the partition-64 denominator rows at partition 0
                nc.scalar.copy(
                    drow[:, hh * 512 : (hh + 1) * 512],
                    upv[hh][DK : DK + 1, :],
                )
            nc.vector.reciprocal_approx_fast(drow[:], drow[:])
            # full-tile broadcast (sliced outputs break on HW)
            bc = bcp.tile([P, S], f32, tag="bc", name="bc")
            nc.gpsimd.partition_broadcast(bc[:], drow[:])
            for hh in range(2):
                base = hh * DK
                nc.vector.tensor_tensor(
                    attnT[base : base + DK, pr, sc * 512 : (sc + 1) * 512],
                    upv[hh][0:DK, :],
                    bc[0:DK, hh * 512 : (hh + 1) * 512],
                    MULT,
                )

        def emit_d_block(pr, sc, expp, bcp, rrowp, defer_hook=None):
            pv_alloc = lambda: [
                pvp.tile([P, 512], f32, tag="pv", name=f"pv{i}")
                for i in range(2)
            ]
            if defer_hook is None:
                pv = pv_alloc()
                for sk in range(ST):
                    ex = emit_scores(pr, sc, sk, expp)
                    emit_pv(pr, sk, ex, pv)
            else:
                exs = [emit_scores(pr, sc, sk, expp) for sk in range(ST)]
                defer_hook()
                pv = pv_alloc()
                for sk in range(ST):
                    emit_pv(pr, sk, exs[sk], pv)
            emit_norm(pr, sc, pv, expp, bcp, rrowp)

        with (
            tc.tile_pool(name="wstage", bufs=2) as wstage,
            tc.tile_pool(name="expp", bufs=7) as expp,
            tc.tile_pool(name="bcp", bufs=2) as bcp,
            tc.tile_pool(name="rrow", bufs=2) as rrowp,
        ):

            # ---------------- phase C: V projection (split by ncx) -------
            def emit_c(ncx):
                for ch in (2 * ncx, 2 * ncx + 1):
                    wv_stage = wstage.tile([P, DT, 256], f32, tag="wvs")
                    nc.sync.dma_start(
                        wv_stage[:],
                        w_in_kp[
                            :, :, 2 * D + ch * 256 : 2 * D + (ch + 1) * 256
                        ],
                    )
                    nc.vector.tensor_copy(
                        wv_tile[:, :, ch * 256 : (ch + 1) * 256], wv_stage[:]
                    )
                for st in range(ST):
                    ps = mps.tile([P, 512], f32, tag="w", name="psc")
                    for dk in range(DT):
                        nc.tensor.matmul(
                            ps[:],
                            xT[:, dk, st * P : (st + 1) * P],
                            wv_tile[:, dk, ncx * 512 : (ncx + 1) * 512],
                            start=(dk == 0),
                            stop=(dk == DT - 1),
                        )
                    nc.vector.tensor_tensor(
                        v_aug[:, st, 8 * ncx : 8 * (ncx + 1), 0:DK],
                        ps[:].rearrange("p (h d) -> p h d", d=DK),
                        bv_bc[:, ncx * 512 : (ncx + 1) * 512].rearrange(
                            "p (h d) -> p h d", d=DK
                        ),
                        ADD,
                    )

            emit_b(0)
            # D(0,0) scores+exps run during C; its PVs come after C
            emit_d_block(
                0, 0, expp, bcp, rrowp,
                defer_hook=lambda: (emit_c(0), emit_c(1)),
            )
            emit_d_block(0, 1, expp, bcp, rrowp)

            # ---------------- remaining pairs ----------------
            for pr in range(1, PAIRS):
                emit_b(pr)
                for sc in range(SC):
                    emit_d_block(pr, sc, expp, bcp, rrowp)

            # W_out staged load + bf16 cast (overlaps the pair loop)
            for ch in range(4):
                wo_stage = wstage.tile([P, DT, 256], f32, tag="wvs")
                nc.sync.dma_start(
                    wo_stage[:],
                    w_out_kp[:, :, ch * 256 : (ch + 1) * 256],
                )
                nc.vector.tensor_copy(
                    wout_bf[:, :, ch * 256 : (ch + 1) * 256], wo_stage[:]
                )

        # ---------------- phase E: output projection (bf16) ----------------
        with tc.tile_pool(name="ypool", bufs=2) as ypool:
            for st in range(ST):
                ps = [
                    mps.tile([P, 512], f32, tag="w", name=f"pse{ncx}")
                    for ncx in range(SC)
                ]
                for dk in range(DT):
                    for ncx in range(SC):
                        nc.tensor.matmul(
                            ps[ncx][:],
                            attnT[:, dk, st * P : (st + 1) * P],
                            wout_bf[:, dk, ncx * 512 : (ncx + 1) * 512],
                            start=(dk == 0),
                            stop=(dk == DT - 1),
                        )
                y = ypool.tile([P, D], f32, tag="y")
                for ncx in range(SC):
                    nc.vector.tensor_tensor(
                        y[:, ncx * 512 : (ncx + 1) * 512],
                        ps[ncx][:],
                        bout_bc[:, ncx * 512 : (ncx + 1) * 512],
                        ADD,
                    )
                nc.sync.dma_start(out[st * P : (st + 1) * P, :], y[:])

        for pool in (
            wqk,
            pa,
            wv_pool,
            woutb_pool,
            attnT_pool,
            xT_pool,
            vaug_pool,
            qkT_pool,
            pvp,
            dwide,
            mps,
            const,
        ):
            pool.release()

    nc.finalize()
    return nc


_NC_CACHE = {}


def get_nc():
    if "nc" not in _NC_CACHE:
        _NC_CACHE["nc"] = build_nc()
    return _NC_CACHE["nc"]


def kernel(X, W_in, b_in, W_out, b_out):
    X = np.ascontiguousarray(np.asarray(X, dtype=np.float32))
    W_in = np.ascontiguousarray(np.asarray(W_in, dtype=np.float32))
    b_in = np.ascontiguousarray(np.asarray(b_in, dtype=np.float32))
    W_out = np.ascontiguousarray(np.asarray(W_out, dtype=np.float32))
    b_out = np.ascontiguousarray(np.asarray(b_out, dtype=np.float32))

    nc = get_nc()
    in_maps = [
        {"X": X[i], "W_in": W_in, "b_in": b_in, "W_out": W_out, "b_out": b_out}
        for i in range(B)
    ]
    res = run_bass_kernel_spmd(nc, in_maps, core_ids=list(range(B)))
    return np.stack([res.results[i]["out"] for i in range(B)], axis=0)


# revision 35
# speedup vs baseline: 1.0511x; 1.0118x over previous
"""Multi-head attention (B=8, S=1024, D=1024, H=16) on 8 TRN2 NeuronCores.

Sharding: pure data parallel - batch element b on core b. Weights are
broadcast to every core. No collectives.

Per-core pipeline (X: [S, D] for one batch element), restructured so the
softmax exp (the ACT-engine bottleneck) overlaps PE matmul work:

  A. X^T via PE transposes (bf16 matmul against identity), 4 blocks per
     PSUM bank, one wide ACT copy per 4 blocks.
  B(pr). Q^T/K^T projection for head pair pr only (nt = pr and 8+pr),
     dk-outer / sc-inner loop so each LDWEIGHTS feeds 2 matmuls; bias
     (+psum->bf16 copy) on DVE to keep ACT free for exp.
  C. V = X @ W_v (natural layout) into V_aug[sk, head, 65] with a ones
     column (col 64) that makes the PV matmul accumulate the softmax
     denominator in row 64.
  D(pr). attention for the pair: scores^T = K_h^T.T @ Q_h^T into a wide
     [128,1024] PSUM tile (two heads row-split, concurrent on the PE),
     exp split 6:2 between ACT (exact, scale=1/8) and DVE (Schraudolph
     int16 bit-trick writing bf16 bit patterns), PV with the ones row.
     Normalize is decoupled from the PSUM path: one bf16 copy per head
     frees the pv bank, then reciprocal_approx_fast (from a partition-0
     staging row - custom-DVE ops mis-read nonzero base partitions on
     HW) + GpSimd partition-broadcast + DVE multiply -> attnT bf16.
  E. Y = attn_out @ W_out + b_out in bf16.

Emission order A; B(0); D(0,0){scores; C as defer hook; PV}; D(0,1);
B(1); D(1); ... B(7); D(7); E lets the Tile scheduler run B(pr+1)
matmuls in the PE gaps of D(pr) while ACT/DVE stream the exps, and run
D(0)'s scores/exps during C. PSUM: 2x[128,512] misc + 2x[128,1024]
scores + 2x[128,512] pv = 8 banks.
"""

import sys

sys.path.insert(0, "/opt/trn_rl_repo")

import numpy as np

import concourse.bacc as bacc
import concourse.mybir as mybir
from concourse.bass_utils import run_bass_kernel_spmd
from concourse.masks import make_identity
from concourse.tile import TileContext

B = 8
S = 1024
D = 1024
H = 16
DK = D // H  # 64
P = 128
ST = S // P   # 8 s-tiles
DT = D // P   # 8 d-tiles
NTQK = 2 * D // P  # 16 n-tiles for the Q|K part
PAIRS = H // 2     # 8 head pairs
SC = S // 512      # 2 chunks of 512 (matmul free-dim limit)

f32 = mybir.dt.float32
bf16 = mybir.dt.bfloat16
i16 = mybir.dt.int16
EXP = mybir.ActivationFunctionType.Exp
MULT = mybir.AluOpType.mult
ADD = mybir.AluOpType.add

# Schraudolph exp on DVE: bf16 bits of exp(s/8) = int16((a/(8*65536))*s + b)
# with a = 2^23/ln2, b = (127*2^23 - C)/65536, C = 366393 (min max-rel-err).
SCH_A = float(2.0**23 / np.log(2.0) / (8.0 * 65536.0))
SCH_B = float((127 * 2.0**23 - 366393.0) / 65536.0)
# which sk-tiles of each (pair, sc) block use the DVE exp (rest use ACT)
DVE_EXP_SKS = frozenset({3, 7})


def build_nc():
    nc = bacc.Bacc()
    X = nc.dram_tensor("X", [S, D], f32, kind="ExternalInput")
    W_in = nc.dram_tensor("W_in", [D, 3 * D], f32, kind="ExternalInput")
    b_in = nc.dram_tensor("b_in", [3 * D], f32, kind="ExternalInput")
    W_out = nc.dram_tensor("W_out", [D, D], f32, kind="ExternalInput")
    b_out = nc.dram_tensor("b_out", [D], f32, kind="ExternalInput")
    out = nc.dram_tensor("out", [S, D], f32, kind="ExternalOutput")

    w_in_kp = W_in.rearrange("(ko p) n -> p ko n", p=P)  # [128, 8, 3072]
    w_out_kp = W_out.rearrange("(ko p) n -> p ko n", p=P)  # [128, 8, 1024]

    with TileContext(nc) as tc:
        const = tc.alloc_tile_pool(name="const", bufs=1)
        # PSUM: mps 2x[128,512] (2 banks) shared by A/B/C/E, dwide
        # 2x[128,1024] (4 banks) for scores, pvp 2x[128,512] (2 banks).
        mps = tc.alloc_tile_pool(name="mps", bufs=2, space="PSUM")
        dwide = tc.alloc_tile_pool(name="dwide", bufs=2, space="PSUM")
        pvp = tc.alloc_tile_pool(name="pvp", bufs=2, space="PSUM")

        identity = const.tile([P, P], bf16)
        make_identity(nc, identity[:])
        # HAM warm-up: keep the PE busy through the DMA-bound head so the
        # real matmuls start at the full 2.4 GHz clock (K=8/8). The sink DMA
        # keeps DCE from dropping the chain.
        warm_scratch = nc.dram_tensor(
            "warm_scratch", [1, 8], f32, kind="Internal"
        )
        warm_sink = const.tile([1, 8], f32)
        bqk = const.tile([P, NTQK], f32)
        nc.sync.dma_start(bqk[:], b_in[0 : 2 * D].rearrange("(o p) -> p o", p=P))
        bv_bc = const.tile([P, D], f32)
        bout_bc = const.tile([P, D], f32)
        ones4 = const.tile([P, ST, H, 1], f32)
        nc.vector.memset(ones4[:], 1.0)

        # ---------------- resident tensors ----------------
        qkT_pool = tc.alloc_tile_pool(name="qkT", bufs=1)
        qkT = qkT_pool.tile([P, NTQK, S], bf16)  # 4 MB
        vaug_pool = tc.alloc_tile_pool(name="vaug", bufs=1)
        v_aug = vaug_pool.tile([P, ST, H, DK + 1], bf16)  # 2.1 MB
        nc.vector.tensor_copy(v_aug[:, :, :, DK : DK + 1], ones4[:])
        xT_pool = tc.alloc_tile_pool(name="xT", bufs=1)
        xT = xT_pool.tile([P, DT, S], bf16)  # 2 MB
        attnT_pool = tc.alloc_tile_pool(name="attnT", bufs=1)
        attnT = attnT_pool.tile([P, DT, S], bf16)  # 2 MB
        woutb_pool = tc.alloc_tile_pool(name="woutb", bufs=1)
        wout_bf = woutb_pool.tile([P, DT, D], bf16)  # 2 MB
        wv_pool = tc.alloc_tile_pool(name="wv", bufs=1)
        wv_tile = wv_pool.tile([P, DT, D], bf16)  # 2 MB

        pa = tc.alloc_tile_pool(name="phaseA", bufs=1)
        bv_row = pa.tile([1, D], f32)
        nc.sync.dma_start(bv_row[:], b_in[None, 2 * D : 3 * D])
        nc.gpsimd.partition_broadcast(bv_bc[:], bv_row[:])
        bout_row = pa.tile([1, D], f32)
        nc.sync.dma_start(bout_row[:], b_out[None, :])
        nc.gpsimd.partition_broadcast(bout_bc[:], bout_row[:])

        wtp = mps.tile([P, 512], f32, tag="w", name="warm")
        for wi in range(64):
            nc.tensor.matmul(
                wtp[:, 0:P],
                identity[:],
                identity[:],
                start=(wi == 0),
                stop=(wi == 63),
            )
        nc.vector.tensor_copy(warm_sink[:], wtp[0:1, 0:8])
        nc.sync.dma_start(warm_scratch[:], warm_sink[:])

        wqk = tc.alloc_tile_pool(name="wqk", bufs=3)

        def emit_w_dma(nt):
            w_stage = wqk.tile([P, DT, P], f32, tag="ws")
            nc.sync.dma_start(w_stage[:], w_in_kp[:, :, nt * P : (nt + 1) * P])
            return w_stage

        # ---------------- phase A: X^T (PE transpose, cast to bf16) --------
        w_pref = {}
        with tc.tile_pool(name="xstage", bufs=3) as xstage:
            for si in range(ST):
                x_tile = xstage.tile([P, D], f32, tag="x")
                nc.sync.dma_start(x_tile[:], X[si * P : (si + 1) * P, :])
                if si == 0:
                    # prefetch pair-0 QK weights right after the first X tile
                    w_pref = {0: emit_w_dma(0), PAIRS: emit_w_dma(PAIRS)}
                xb = xstage.tile([P, D], bf16, tag="xb")
                nc.vector.tensor_copy(xb[:], x_tile[:])
                for half in range(2):
                    tp = mps.tile([P, 512], f32, tag="w", name="tp")
                    for j in range(4):
                        dj = half * 4 + j
                        nc.tensor.matmul(
                            tp[:, j * P : (j + 1) * P],
                            xb[:, dj * P : (dj + 1) * P],
                            identity[:],
                            start=True,
                            stop=True,
                        )
                    nc.scalar.copy(
                        xT[:, half * 4 : (half + 1) * 4, si * P : (si + 1) * P],
                        tp[:].rearrange("p (j q) -> p j q", q=P),
                    )

        # ---------------- per-pair B (QK projection) ----------------
        def emit_b(pr):
            for nt in (pr, PAIRS + pr):
                w_stage = w_pref.pop(nt, None)
                if w_stage is None:
                    w_stage = emit_w_dma(nt)
                w_tile = wqk.tile([P, DT, P], bf16, tag="w")
                nc.vector.tensor_copy(w_tile[:], w_stage[:])
                ps = [
                    mps.tile([P, 512], f32, tag="w", name=f"psb{sc}")
                    for sc in range(SC)
                ]
                for dk in range(DT):
                    for sc in range(SC):
                        nc.tensor.matmul(
                            ps[sc][:],
                            w_tile[:, dk, :],
                            xT[:, dk, sc * 512 : (sc + 1) * 512],
                            start=(dk == 0),
                            stop=(dk == DT - 1),
                        )
                for sc in range(SC):
                    nc.vector.tensor_scalar(
                        qkT[:, nt, sc * 512 : (sc + 1) * 512],
                        ps[sc][:],
                        bqk[:, nt : nt + 1],
                        None,
                        ADD,
                    )

        # ---------------- per-pair D (attention) ----------------
        def emit_scores(pr, sc, sk, expp):
            wide = dwide.tile([P, S], f32, tag="sps", name="sps")
            for hh in range(2):
                base = hh * DK
                nc.tensor.matmul(
                    wide[:, hh * 512 : (hh + 1) * 512],
                    qkT[base : base + DK, PAIRS + pr, sk * P : (sk + 1) * P],
                    qkT[base : base + DK, pr, sc * 512 : (sc + 1) * 512],
                    start=True,
                    stop=True,
                )
            ex = expp.tile([P, S], bf16, tag="ex")
            if sk in DVE_EXP_SKS:
                nc.vector.tensor_scalar(
                    ex[:].bitcast(i16), wide[:], SCH_A, SCH_B, MULT, ADD
                )
            else:
                nc.scalar.activation(
                    ex[:], wide[:], EXP, scale=1.0 / np.sqrt(DK)
                )
            return ex

        def emit_pv(pr, sk, ex, pv):
            for hh in range(2):
                h = 2 * pr + hh
                nc.tensor.matmul(
                    pv[hh][0 : DK + 1, :],
                    v_aug[:, sk, h, :],
                    ex[:, hh * 512 : (hh + 1) * 512],
                    start=(sk == 0),
                    stop=(sk == ST - 1),
                )

        def emit_norm(pr, sc, pv, expp, bcp, rrowp):
            # Decouple normalization from the PSUM critical path: one bf16
            # copy per head frees the pv bank (values + denominator row);
            # the reciprocal chain then runs off-path from SBUF.
            upv = [
                rrowp.tile([P, 512], bf16, tag=f"upv{hh}", name=f"upv{hh}")
                for hh in range(2)
            ]
            for hh in range(2):
                nc.vector.tensor_copy(upv[hh][0 : DK + 1, :], pv[hh][0 : DK + 1, :])
            drow = rrowp.tile([1, S], f32, tag="dr", name="drow")
            for hh in range(2):
                # custom-DVE ops mis-read nonzero base_partition on HW:
                # stage the partition-64 denominator rows at partition 0
                nc.scalar.copy(
                    drow[:, hh * 512 : (hh + 1) * 512],
                    upv[hh][DK : DK + 1, :],
                )
            nc.vector.reciprocal_approx_fast(drow[:], drow[:])
            # full-tile broadcast (sliced outputs break on HW)
            bc = bcp.tile([P, S], f32, tag="bc", name="bc")
            nc.gpsimd.partition_broadcast(bc[:], drow[:])
            for hh in range(2):
                base = hh * DK
                nc.vector.tensor_tensor(
                    attnT[base : base + DK, pr, sc * 512 : (sc + 1) * 512],
                    upv[hh][0:DK, :],
                    bc[0:DK, hh * 512 : (hh + 1) * 512],
                    MULT,
                )

        def emit_d_block(pr, sc, expp, bcp, rrowp, defer_hook=None):
            pv_alloc = lambda: [
                pvp.tile([P, 512], f32, tag="pv", name=f"pv{i}")
                for i in range(2)
            ]
            if defer_hook is None:
                pv = pv_alloc()
                for sk in range(ST):
                    ex = emit_scores(pr, sc, sk, expp)
                    emit_pv(pr, sk, ex, pv)
            else:
                exs = [emit_scores(pr, sc, sk, expp) for sk in range(ST)]
                defer_hook()
                pv = pv_alloc()
                for sk in range(ST):
                    emit_pv(pr, sk, exs[sk], pv)
            emit_norm(pr, sc, pv, expp, bcp, rrowp)

        with (
            tc.tile_pool(name="wstage", bufs=2) as wstage,
            tc.tile_pool(name="expp", bufs=7) as expp,
            tc.tile_pool(name="bcp", bufs=2) as bcp,
            tc.tile_pool(name="rrow", bufs=2) as rrowp,
        ):

            # ---------------- phase C: V projection (split by ncx) -------
            def emit_c(ncx):
                for ch in (2 * ncx, 2 * ncx + 1):
                    wv_stage = wstage.tile([P, DT, 256], f32, tag="wvs")
                    nc.sync.dma_start(
                        wv_stage[:],
                        w_in_kp[
                            :, :, 2 * D + ch * 256 : 2 * D + (ch + 1) * 256
                        ],
                    )
                    nc.vector.tensor_copy(
                        wv_tile[:, :, ch * 256 : (ch + 1) * 256], wv_stage[:]
                    )
                for st in range(ST):
                    ps = mps.tile([P, 512], f32, tag="w", name="psc")
                    for dk in range(DT):
                        nc.tensor.matmul(
                            ps[:],
                            xT[:, dk, st * P : (st + 1) * P],
                            wv_tile[:, dk, ncx * 512 : (ncx + 1) * 512],
                            start=(dk == 0),
                            stop=(dk == DT - 1),
                        )
                    nc.vector.tensor_tensor(
                        v_aug[:, st, 8 * ncx : 8 * (ncx + 1), 0:DK],
                        ps[:].rearrange("p (h d) -> p h d", d=DK),
                        bv_bc[:, ncx * 512 : (ncx + 1) * 512].rearrange(
                            "p (h d) -> p h d", d=DK
                        ),
                        ADD,
                    )

            emit_b(0)
            # D(0,0) scores+exps run during C; its PVs come after C
            emit_d_block(
                0, 0, expp, bcp, rrowp,
                defer_hook=lambda: (emit_c(0), emit_c(1)),
            )
            emit_d_block(0, 1, expp, bcp, rrowp)

            # ---------------- remaining pairs ----------------
            for pr in range(1, PAIRS):
                emit_b(pr)
                for sc in range(SC):
                    emit_d_block(pr, sc, expp, bcp, rrowp)

            # W_out staged load + bf16 cast (overlaps the pair loop)
            for ch in range(4):
                wo_stage = wstage.tile([P, DT, 256], f32, tag="wvs")
                nc.sync.dma_start(
                    wo_stage[:],
                    w_out_kp[:, :, ch * 256 : (ch + 1) * 256],
                )
                nc.vector.tensor_copy(
                    wout_bf[:, :, ch * 256 : (ch + 1) * 256], wo_stage[:]
                )

        # ---------------- phase E: output projection (bf16) ----------------
        with tc.tile_pool(name="ypool", bufs=2) as ypool:
            for st in range(ST):
                ps = [
                    mps.tile([P, 512], f32, tag="w", name=f"pse{ncx}")
                    for ncx in range(SC)
                ]
                for dk in range(DT):
                    for ncx in range(SC):
                        nc.tensor.matmul(
                            ps[ncx][:],
                            attnT[:, dk, st * P : (st + 1) * P],
                            wout_bf[:, dk, ncx * 512 : (ncx + 1) * 512],
                            start=(dk == 0),
                            stop=(dk == DT - 1),
                        )
                y = ypool.tile([P, D], f32, tag="y")
                for ncx in range(SC):
                    nc.vector.tensor_tensor(
                        y[:, ncx * 512 : (ncx + 1) * 512],
                        ps[ncx][:],
                        bout_bc[:, ncx * 512 : (ncx + 1) * 512],
                        ADD,
                    )
                nc.sync.dma_start(out[st * P : (st + 1) * P, :], y[:])

        for pool in (
            wqk,
            pa,
            wv_pool,
            woutb_pool,
            attnT_pool,
            xT_pool,
            vaug_pool,
            qkT_pool,
            pvp,
            dwide,
            mps,
            const,
        ):
            pool.release()

    nc.finalize()
    return nc


_NC_CACHE = {}


def get_nc():
    if "nc" not in _NC_CACHE:
        _NC_CACHE["nc"] = build_nc()
    return _NC_CACHE["nc"]


def kernel(X, W_in, b_in, W_out, b_out):
    X = np.ascontiguousarray(np.asarray(X, dtype=np.float32))
    W_in = np.ascontiguousarray(np.asarray(W_in, dtype=np.float32))
    b_in = np.ascontiguousarray(np.asarray(b_in, dtype=np.float32))
    W_out = np.ascontiguousarray(np.asarray(W_out, dtype=np.float32))
    b_out = np.ascontiguousarray(np.asarray(b_out, dtype=np.float32))

    nc = get_nc()
    in_maps = [
        {"X": X[i], "W_in": W_in, "b_in": b_in, "W_out": W_out, "b_out": b_out}
        for i in range(B)
    ]
    res = run_bass_kernel_spmd(nc, in_maps, core_ids=list(range(B)))
    return np.stack([res.results[i]["out"] for i in range(B)], axis=0)
